# revision 8
# baseline (speedup 1.0000x reference)
"""Distributed Trainium2 Bass kernel for nn_Attention_50139448213963.

Attention layer with per-head QK-layernorm + interleaved RoPE:
  qkv = x @ Wqkv_w.T + Wqkv_b ; q,k = LN_head(q|k) ; q,k = rope(q|k)
  out = softmax(q k^T / sqrt(d)) v ; out = concat_heads @ out_w.T + out_b

Sharding (8 cores): core c -> batch c//4, heads {2*(c%4), 2*(c%4)+1}.
Each core computes QKV for its 2 heads, attention, and the out-proj
partial using its heads' columns of out_w.  Host sums the 4 partials
per batch and adds the (host-foldable) bias terms.

Per-core dataflow (all matmuls bf16, fp32 accumulation):
  1. QKV in normal layout [tok, outdim]:  lhsT = xT k-chunk, rhs = w chunk.
  2. LN on q,k read straight from PSUM (bn_stats/bn_aggr + fused
     (x-mu)*rs tensor_scalar).  RoPE with head_dim PERMUTED even-first so
     rotate_half becomes contiguous 64-block swaps; q_gamma/k_gamma and
     the 1/sqrt(d) score scale are folded into host-built cos/sin tables.
  3. q,k DMA-transposed to [d, tok] bf16.
  4. Per (head, 512-wide q chunk): scoresT = kT_tile^T @ qT  (16 k-tiles),
     exp on ScalarE (PSUM->SBUF bf16), softmax sums via ones-matmul,
     AV accumulated over k-tiles -> avT [d, 512] PSUM.
  5. Normalize: sums -> reciprocal -> rank-1 broadcast matmul -> TT mul,
     avT_norm written bf16.
  6. Out-proj: per tok-tile, accumulate both heads' W-chunks, evict fp32,
     DMA out [2048, 1024] partial.
"""

import math
import os
from contextlib import ExitStack

import numpy as np
import ml_dtypes

import concourse.bass as bass
import concourse.tile as tile
from concourse import bacc, mybir
from concourse.bass import ts, ds
from concourse.bass_utils import run_bass_kernel_spmd

F32 = mybir.dt.float32
BF16 = mybir.dt.bfloat16

DIM = 1024
HEADS = 8
D = 128  # head dim
B = 2
N = 2048
EPS = 1e-6
HPC = 2  # heads per core
N_CORES = 8
P = 128  # partitions
QC = 512  # q chunk for attention
N_TILES = N // P  # 16
K_IN = DIM // P  # 8 k-tiles over input dim
W_OUT = HPC * 3 * D  # 768 qkv outdims per core
# wqkv block layout (free offsets): q0,k0,q1,k1 then v0,v1
OFF_Q = [0 * D, 2 * D]
OFF_K = [1 * D, 3 * D]
OFF_V = [4 * D, 5 * D]


def build_core_graph(nc, n_tok=N, dtype_mm=BF16, emit_qk_bias=False):
    """Emit the per-core program. All cores run the same graph (SPMD)."""
    n_tiles = n_tok // P
    n_qc = n_tok // QC if n_tok >= QC else 1
    qc = min(QC, n_tok)

    # ---- dram parameters ----
    xT = nc.dram_tensor("xT", [DIM, n_tok], dtype_mm, kind="ExternalInput").ap()
    wqkv = nc.dram_tensor("wqkv", [DIM, W_OUT], dtype_mm, kind="ExternalInput").ap()
    wout = nc.dram_tensor("wout", [HPC * D, DIM], dtype_mm, kind="ExternalInput").ap()
    ropeq = nc.dram_tensor("ropeq", [n_tok, 2 * HPC * D], F32, kind="ExternalInput").ap()
    ropek = nc.dram_tensor("ropek", [n_tok, 2 * HPC * D], F32, kind="ExternalInput").ap()
    bqkv = None
    if emit_qk_bias:
        bqkv = nc.dram_tensor("bqkv", [1, W_OUT], F32, kind="ExternalInput").ap()
    out = nc.dram_tensor("out", [n_tok, DIM], F32, kind="ExternalOutput").ap()

    with tile.TileContext(nc) as tc, ExitStack() as ctx:
        const = ctx.enter_context(tc.tile_pool(name="const", bufs=1))
        big = ctx.enter_context(tc.tile_pool(name="big", bufs=1))

        # resident SBUF tensors
        xT_sb = big.tile([P, K_IN, n_tok], dtype_mm, tag="xT_sb")
        wqkv_sb = big.tile([P, K_IN, W_OUT], dtype_mm, tag="wqkv_sb")
        wout_sb = big.tile([P, HPC, DIM], dtype_mm, tag="wout_sb")
        qT_sb = [big.tile([P, n_tok], dtype_mm, tag=f"qT{h}", name=f"qT{h}")
                 for h in range(HPC)]
        kT_sb = [big.tile([P, n_tok], dtype_mm, tag=f"kT{h}", name=f"kT{h}")
                 for h in range(HPC)]
        v_sb = [big.tile([P, n_tiles, D], dtype_mm, tag=f"v{h}", name=f"v{h}")
                for h in range(HPC)]
        avn_sb = [big.tile([P, n_tok], dtype_mm, tag=f"avn{h}", name=f"avn{h}")
                  for h in range(HPC)]

        ones_col = const.tile([P, 1], dtype_mm)  # sums-matmul lhsT
        nc.vector.memset(ones_col[:], 1.0)
        ones_row = const.tile([1, P], F32)  # bcast rank-1 lhsT
        nc.vector.memset(ones_row[:], 1.0)
        eps_col = const.tile([P, 1], F32)
        nc.vector.memset(eps_col[:], EPS)

        # load weights/activations
        for kk in range(K_IN):
            nc.sync.dma_start(xT_sb[:, kk, :], xT[ts(kk, P), :])
            nc.sync.dma_start(wqkv_sb[:, kk, :], wqkv[ts(kk, P), :])
        for h in range(HPC):
            nc.sync.dma_start(wout_sb[:, h, :], wout[ts(h, P), :])

        if emit_qk_bias:
            bias_sb = const.tile([1, W_OUT], F32)
            nc.sync.dma_start(bias_sb[:], bqkv[:])

        # ---------- stage 1: QKV + LN + RoPE ----------
        stage1 = ExitStack()
        qkv_ps = stage1.enter_context(tc.tile_pool(name="qkv_ps", bufs=2, space="PSUM"))
        rope_pool = stage1.enter_context(tc.tile_pool(name="rope", bufs=3))
        s1 = stage1.enter_context(tc.tile_pool(name="s1", bufs=3))
        s1small = stage1.enter_context(tc.tile_pool(name="s1small", bufs=4))

        for t in range(n_tiles):
            # two psum chunks: [q0,k0,q1,k1] (512) and [v0,v1] (256)
            ps_a = qkv_ps.tile([P, 4 * D], F32, tag="ps_a")
            ps_b = qkv_ps.tile([P, 2 * D], F32, tag="ps_b")
            for kk in range(K_IN):
                lhsT = xT_sb[:, kk, ts(t, P)]
                nc.tensor.matmul(ps_a[:], lhsT, wqkv_sb[:, kk, 0:4 * D],
                                 start=(kk == 0), stop=(kk == K_IN - 1))
                nc.tensor.matmul(ps_b[:], lhsT, wqkv_sb[:, kk, 4 * D:W_OUT],
                                 start=(kk == 0), stop=(kk == K_IN - 1))
            if emit_qk_bias:
                # += ones[tok] x bias  (rank-1)
                nc.tensor.matmul(ps_a[:], ones_row[:, :], bias_sb[:, 0:4 * D],
                                 start=False, stop=True)
                nc.tensor.matmul(ps_b[:], ones_row[:, :], bias_sb[:, 4 * D:W_OUT],
                                 start=False, stop=True)

            # rope tables for this tile
            rq = rope_pool.tile([P, 2 * HPC * D], F32, tag="rq")
            rk = rope_pool.tile([P, 2 * HPC * D], F32, tag="rk")
            nc.sync.dma_start(rq[:], ropeq[ts(t, P), :])
            nc.sync.dma_start(rk[:], ropek[ts(t, P), :])

            # v: straight cast-evict to [tok, d] bf16
            for h in range(HPC):
                nc.scalar.activation(v_sb[h][:, t, :], ps_b[:, ts(h, D)],
                                     mybir.ActivationFunctionType.Copy)

            # LN stats for q0,k0,q1,k1
            stats = s1small.tile([P, 4, 6], F32, tag="stats")
            mv = s1small.tile([P, 4, 2], F32, tag="mv")
            rs = s1small.tile([P, 4], F32, tag="rs")
            for s in range(4):
                nc.vector.bn_stats(stats[:, s, :], ps_a[:, ts(s, D)])
                nc.vector.bn_aggr(mv[:, s, :], stats[:, s, :])
            # rs = 1/sqrt(var+eps)
            sd = s1small.tile([P, 4], F32, tag="sd")
            nc.scalar.activation(sd[:, :], mv[:, :, 1],
                                 mybir.ActivationFunctionType.Sqrt,
                                 bias=eps_col[:])
            nc.vector.reciprocal(rs[:, :], sd[:, :])

            qn = s1.tile([P, HPC * D], F32, tag="qn")
            kn = s1.tile([P, HPC * D], F32, tag="kn")
            # (x - mu) * rs  in one tensor_scalar per slice
            for h in range(HPC):
                nc.vector.tensor_scalar(qn[:, ts(h, D)], ps_a[:, ts(2 * h, D)],
                                        mv[:, 2 * h, 0:1], rs[:, 2 * h:2 * h + 1],
                                        mybir.AluOpType.subtract,
                                        mybir.AluOpType.mult)
                nc.vector.tensor_scalar(kn[:, ts(h, D)], ps_a[:, ts(2 * h + 1, D)],
                                        mv[:, 2 * h + 1, 0:1], rs[:, 2 * h + 1:2 * h + 2],
                                        mybir.AluOpType.subtract,
                                        mybir.AluOpType.mult)

            # rotate_half in permuted (even-first) space: block swap
            for (xn, rt, name) in ((qn, rq, "q"), (kn, rk, "k")):
                rot = s1.tile([P, HPC * D], F32, tag=f"rot{name}")
                x4 = xn.rearrange("p (s x) -> p s x", x=D)
                r4 = rot.rearrange("p (s x) -> p s x", x=D)
                nc.vector.tensor_scalar_mul(r4[:, :, 0:D // 2], x4[:, :, D // 2:D], -1.0)
                nc.vector.tensor_copy(r4[:, :, D // 2:D], x4[:, :, 0:D // 2])
                # x*cos + rot*sin  (cos table in rt[:, 0:256], sin in rt[:, 256:512])
                a = s1.tile([P, HPC * D], F32, tag=f"a{name}")
                bb = s1.tile([P, HPC * D], F32, tag=f"b{name}")
                nc.vector.tensor_tensor(a[:], xn[:], rt[:, 0:HPC * D],
                                        mybir.AluOpType.mult)
                nc.vector.tensor_tensor(bb[:], rot[:], rt[:, HPC * D:2 * HPC * D],
                                        mybir.AluOpType.mult)
                ro = s1.tile([P, HPC * D], dtype_mm, tag=f"ro{name}")
                nc.vector.tensor_tensor(ro[:], a[:], bb[:], mybir.AluOpType.add)
                # transpose [tok,d] -> [d,tok] per head via DMA xbar
                dst = qT_sb if name == "q" else kT_sb
                for h in range(HPC):
                    nc.sync.dma_start(dst[h][:, ts(t, P)], ro[:, ts(h, D)],
                                      transpose=True)

        stage1.close()

        # ---------- stage 2: attention ----------
        stage2 = ExitStack()
        sc_ps = stage2.enter_context(tc.tile_pool(name="sc_ps", bufs=2, space="PSUM"))
        av_ps = stage2.enter_context(tc.tile_pool(name="av_ps", bufs=2, space="PSUM"))
        sum_ps = stage2.enter_context(tc.tile_pool(name="sum_ps", bufs=1, space="PSUM"))
        bc_ps = stage2.enter_context(tc.tile_pool(name="bc_ps", bufs=1, space="PSUM"))
        probs = stage2.enter_context(tc.tile_pool(name="probs", bufs=4))
        s2 = stage2.enter_context(tc.tile_pool(name="s2", bufs=3))

        for qi in range(n_qc):
            for h in range(HPC):
                av = av_ps.tile([P, qc], F32, tag="av")
                sums = sum_ps.tile([1, qc], F32, tag="sums")
                for kt in range(n_tiles):
                    sc = sc_ps.tile([P, qc], F32, tag="sc")
                    nc.tensor.matmul(sc[:], kT_sb[h][:, ts(kt, P)],
                                     qT_sb[h][:, ds(qi * qc, qc)],
                                     start=True, stop=True)
                    pr = probs.tile([P, qc], dtype_mm, tag="pr")
                    nc.scalar.activation(pr[:], sc[:],
                                         mybir.ActivationFunctionType.Exp)
                    nc.tensor.matmul(sums[:], ones_col[:], pr[:],
                                     start=(kt == 0), stop=(kt == n_tiles - 1))
                    nc.tensor.matmul(av[:], v_sb[h][:, kt, :], pr[:],
                                     start=(kt == 0), stop=(kt == n_tiles - 1))
                # normalize: recip(sums) -> broadcast -> multiply
                sums_sb = s2.tile([1, qc], F32, tag="sums_sb")
                nc.scalar.activation(sums_sb[:], sums[:],
                                     mybir.ActivationFunctionType.Copy)
                rcp = s2.tile([1, qc], F32, tag="rcp")
                nc.vector.reciprocal(rcp[:], sums_sb[:])
                bc = bc_ps.tile([P, qc], F32, tag="bc")
                nc.tensor.matmul(bc[:], ones_row[:], rcp[:], start=True, stop=True)
                bc_sb = s2.tile([P, qc], F32, tag="bc_sb")
                nc.scalar.activation(bc_sb[:], bc[:],
                                     mybir.ActivationFunctionType.Copy)
                nc.vector.tensor_tensor(avn_sb[h][:, ds(qi * qc, qc)], av[:], bc_sb[:],
                                        mybir.AluOpType.mult)

        stage2.close()

        # ---------- stage 3: out-projection partial ----------
        o_ps = ctx.enter_context(tc.tile_pool(name="o_ps", bufs=2, space="PSUM"))
        s3 = ctx.enter_context(tc.tile_pool(name="s3", bufs=3))
        for t in range(n_tiles):
            for c in range(DIM // QC):
                po = o_ps.tile([P, QC], F32, tag="po")
                for h in range(HPC):
                    nc.tensor.matmul(po[:], avn_sb[h][:, ts(t, P)],
                                     wout_sb[:, h, ts(c, QC)],
                                     start=(h == 0), stop=(h == HPC - 1))
                ot = s3.tile([P, QC], F32, tag="ot")
                nc.scalar.activation(ot[:], po[:],
                                     mybir.ActivationFunctionType.Copy)
                nc.sync.dma_start(out[ts(t, P), ts(c, QC)], ot[:])

    return nc


# ---------------- host side ----------------

def _prep_core_inputs(x, Wqkv_w, Wqkv_b, q_gamma, k_gamma, out_w,
                      rope_cos, rope_sin, n_tok=N):
    """Build the 8 per-core input dicts (numpy, host-side sharding)."""
    bf = ml_dtypes.bfloat16
    scale = 1.0 / math.sqrt(D)
    # even-first permutation of head_dim and the rope partner map
    perm = np.concatenate([np.arange(0, D, 2), np.arange(1, D, 2)])
    partner = np.concatenate([np.arange(0, D, 2) + 1, np.arange(1, D, 2) - 1])
    # tables in permuted space; gamma folded in; q side also gets 1/sqrt(d)
    cosP = rope_cos[:, perm]
    sinP = rope_sin[:, perm]
    gq, gk = q_gamma, k_gamma
    cos_q = (cosP * gq[perm][None, :]) * scale
    sin_q = (sinP * gq[partner][None, :]) * scale
    cos_k = cosP * gk[perm][None, :]
    sin_k = sinP * gk[partner][None, :]
    # per-tile tables hold both heads side by side: [cos|cos|sin|sin]
    ropeq = np.concatenate([cos_q, cos_q, sin_q, sin_q], axis=1).astype(np.float32)
    ropek = np.concatenate([cos_k, cos_k, sin_k, sin_k], axis=1).astype(np.float32)

    Wr = Wqkv_w.reshape(3, HEADS, D, DIM)
    in_maps = []
    for c in range(N_CORES):
        b = c // 4
        hs = [2 * (c % 4), 2 * (c % 4) + 1]
        xT = np.ascontiguousarray(x[b, :n_tok].T).astype(bf)
        blocks = []
        for h in hs:
            blocks.append(Wr[0, h][perm].T)  # q, dim-permuted  [DIM,128]
            blocks.append(Wr[1, h][perm].T)  # k, dim-permuted
        for h in hs:
            blocks.append(Wr[2, h].T)        # v, natural
        wqkv = np.concatenate(blocks, axis=1).astype(bf)  # [DIM, 768]
        wout = np.concatenate(
            [out_w[:, h * D:(h + 1) * D].T for h in hs], axis=0).astype(bf)  # [256,DIM]
        in_maps.append({
            "xT": xT,
            "wqkv": np.ascontiguousarray(wqkv),
            "wout": np.ascontiguousarray(wout),
            "ropeq": ropeq[:n_tok],
            "ropek": ropek[:n_tok],
        })
    return in_maps


def kernel(x, Wqkv_w, Wqkv_b, q_gamma, q_beta, k_gamma, k_beta,
           out_w, out_b, rope_cos, rope_sin, trace=False, tmpdir=None):
    x = np.asarray(x, np.float32)
    Wqkv_w = np.asarray(Wqkv_w, np.float32)
    Wqkv_b = np.asarray(Wqkv_b, np.float32)
    q_gamma = np.asarray(q_gamma, np.float32)
    q_beta = np.asarray(q_beta, np.float32)
    k_gamma = np.asarray(k_gamma, np.float32)
    k_beta = np.asarray(k_beta, np.float32)
    out_w = np.asarray(out_w, np.float32)
    out_b = np.asarray(out_b, np.float32)
    rope_cos = np.asarray(rope_cos, np.float32)
    rope_sin = np.asarray(rope_sin, np.float32)

    assert np.allclose(q_beta, 0) and np.allclose(k_beta, 0), \
        "nonzero q/k layernorm beta not supported by this kernel build"
    emit_qk_bias = not (np.allclose(Wqkv_b[:DIM], 0) and np.allclose(Wqkv_b[DIM:2 * DIM], 0))

    nc = bacc.Bacc("TRN2", target_bir_lowering=False, debug=False,
                   num_devices=N_CORES)
    build_core_graph(nc, n_tok=N, emit_qk_bias=emit_qk_bias)
    nc.compile()

    in_maps = _prep_core_inputs(x, Wqkv_w, Wqkv_b, q_gamma, k_gamma,
                                out_w, rope_cos, rope_sin)
    if emit_qk_bias:
        for c in range(N_CORES):
            hs = [2 * (c % 4), 2 * (c % 4) + 1]
            bq = Wqkv_b[:DIM].reshape(HEADS, D)
            bk = Wqkv_b[DIM:2 * DIM].reshape(HEADS, D)
            perm = np.concatenate([np.arange(0, D, 2), np.arange(1, D, 2)])
            blocks = [np.zeros(0, np.float32)]
            for h in hs:
                blocks += [bq[h][perm], bk[h][perm]]
            blocks += [np.zeros(2 * D, np.float32)]
            in_maps[c]["bqkv"] = np.concatenate(blocks)[None, :].astype(np.float32)

    res = run_bass_kernel_spmd(nc, in_maps, core_ids=list(range(N_CORES)),
                               trace=trace, tmpdir=tmpdir)
    partials = [np.asarray(r["out"], np.float32) for r in res.results]

    # host gather: sum the 4 head-group partials per batch; fold v-bias + out_b
    bv = Wqkv_b[2 * DIM:]
    bias_row = out_b + bv @ out_w.T  # [DIM]
    outp = np.empty((B, N, DIM), np.float32)
    for b in range(B):
        outp[b] = sum(partials[4 * b:4 * b + 4]) + bias_row[None, :]
    kernel.last_exec_time_ns = res.exec_time_ns
    return outp


# revision 28
# speedup vs baseline: 1.1185x; 1.1185x over previous
"""Distributed Trainium2 Bass kernel for nn_Attention_50139448213963.

Attention layer with per-head QK-layernorm + interleaved RoPE:
  qkv = x @ Wqkv_w.T + Wqkv_b ; q,k = LN_head(q|k) ; q,k = rope(q|k)
  out = softmax(q k^T / sqrt(d)) v ; out = concat_heads @ out_w.T + out_b

Sharding (8 cores): core c -> batch c//4, heads {2*(c%4), 2*(c%4)+1}.
Each core computes QKV for its 2 heads, attention, and the out-proj
partial using its heads' columns of out_w.  Host sums the 4 partials
per batch and adds the (host-foldable) bias terms.

Per-core dataflow (all matmuls bf16, fp32 accumulation):
  1. QKV in normal layout [tok, outdim]:  lhsT = xT k-chunk, rhs = w chunk.
  2. LN on q,k read straight from PSUM (bn_stats/bn_aggr + fused
     (x-mu)*rs tensor_scalar).  RoPE with head_dim PERMUTED even-first so
     rotate_half becomes contiguous 64-block swaps; q_gamma/k_gamma and
     the 1/sqrt(d) score scale are folded into host-built cos/sin tables.
  3. q,k DMA-transposed to [d, tok] bf16.
  4. Per (head, 512-wide q chunk): scoresT = kT_tile^T @ qT  (16 k-tiles),
     exp on ScalarE (PSUM->SBUF bf16), softmax sums via ones-matmul,
     AV accumulated over k-tiles -> avT [d, 512] PSUM.
  5. Normalize: sums -> reciprocal -> rank-1 broadcast matmul -> TT mul,
     avT_norm written bf16.
  6. Out-proj: per tok-tile, accumulate both heads' W-chunks, evict fp32,
     DMA out [2048, 1024] partial.
"""

import math
import os
from contextlib import ExitStack

import numpy as np
import ml_dtypes

import concourse.bass as bass
import concourse.tile as tile
from concourse import bacc, mybir
from concourse.bass import ts, ds
from concourse.bass_utils import run_bass_kernel_spmd
from concourse.masks import make_identity

F32 = mybir.dt.float32
F32R = mybir.dt.float32r
BF16 = mybir.dt.bfloat16

DIM = 1024
HEADS = 8
D = 128  # head dim
B = 2
N = 2048
EPS = 1e-6
HPC = 2  # heads per core
N_CORES = 8
P = 128  # partitions
QC = 512  # q chunk for attention
N_TILES = N // P  # 16
K_IN = DIM // P  # 8 k-tiles over input dim
W_OUT = HPC * 3 * D  # 768 qkv outdims per core
# wqkv block layout (free offsets): q0,k0,q1,k1 then v0,v1
OFF_Q = [0 * D, 2 * D]
OFF_K = [1 * D, 3 * D]
OFF_V = [4 * D, 5 * D]


def build_core_graph(nc, n_tok=N, dtype_mm=BF16, emit_qk_bias=False):
    """Emit the per-core program. All cores run the same graph (SPMD)."""
    n_tiles = n_tok // P
    n_qc = n_tok // QC if n_tok >= QC else 1
    qc = min(QC, n_tok)
    tpq = qc // P  # tok tiles per q chunk

    # ---- dram parameters ----
    xT = nc.dram_tensor("xT", [DIM, n_tok], dtype_mm, kind="ExternalInput").ap()
    wqkv = nc.dram_tensor("wqkv", [DIM, W_OUT], dtype_mm, kind="ExternalInput").ap()
    wout = nc.dram_tensor("wout", [HPC * D, DIM], dtype_mm, kind="ExternalInput").ap()
    ropeq = nc.dram_tensor("ropeq", [n_tok, 2 * HPC * D], BF16, kind="ExternalInput").ap()
    ropek = nc.dram_tensor("ropek", [n_tok, 2 * HPC * D], BF16, kind="ExternalInput").ap()
    bqkv = None
    if emit_qk_bias:
        bqkv = nc.dram_tensor("bqkv", [1, W_OUT], F32, kind="ExternalInput").ap()
    out = nc.dram_tensor("out", [n_tok, DIM], F32, kind="ExternalOutput").ap()

    with tile.TileContext(nc) as tc, ExitStack() as ctx:
        const = ctx.enter_context(tc.tile_pool(name="const", bufs=1))
        big = ctx.enter_context(tc.tile_pool(name="big", bufs=1))

        # resident SBUF tensors
        xT_sb = big.tile([P, K_IN, n_tok], dtype_mm, tag="xT_sb")
        wqkv_sb = big.tile([P, K_IN, W_OUT], dtype_mm, tag="wqkv_sb")
        wout_sb = big.tile([P, HPC, DIM], dtype_mm, tag="wout_sb")
        qT_sb = [big.tile([P, n_tok], dtype_mm, tag=f"qT{h}", name=f"qT{h}")
                 for h in range(HPC)]
        kT_sb = [big.tile([P, n_tok], dtype_mm, tag=f"kT{h}", name=f"kT{h}")
                 for h in range(HPC)]
        # v low halves + ones column (row 64 of AV psum = softmax sums), v high
        v1_all = big.tile([P, n_tiles, HPC, D // 2 + 1], dtype_mm, tag="v1_all")
        v2_all = big.tile([P, n_tiles, HPC, D // 2], dtype_mm, tag="v2_all")
        avn_sb = [big.tile([P, qc], dtype_mm, tag=f"avn{h}", name=f"avn{h}", bufs=min(2, n_qc))
                  for h in range(HPC)]

        ones_row = const.tile([1, P], F32)  # bcast rank-1 lhsT
        nc.vector.memset(ones_row[:], 1.0)
        nc.vector.memset(v1_all[:, :, :, D // 2], 1.0)
        eps_col = const.tile([P, 1], F32)
        nc.vector.memset(eps_col[:], EPS)

        ropeq_sb = big.tile([P, n_tiles, 2 * HPC * D], BF16, tag="ropeq_sb")
        ropek_sb = big.tile([P, n_tiles, 2 * HPC * D], BF16, tag="ropek_sb")

        # load weights/activations
        for kk in range(K_IN):
            nc.sync.dma_start(xT_sb[:, kk, :], xT[ts(kk, P), :])
            nc.sync.dma_start(wqkv_sb[:, kk, :], wqkv[ts(kk, P), :])
        for h in range(HPC):
            nc.sync.dma_start(wout_sb[:, h, :], wout[ts(h, P), :])
        nc.gpsimd.dma_start(ropeq_sb[:],
                            ropeq.rearrange("(t p) f -> p t f", p=P))
        nc.gpsimd.dma_start(ropek_sb[:],
                            ropek.rearrange("(t p) f -> p t f", p=P))

        if emit_qk_bias:
            bias_sb = const.tile([1, W_OUT], F32)
            nc.sync.dma_start(bias_sb[:], bqkv[:])

        # ---------- stage 1: QKV + LN + RoPE ----------
        ident = const.tile([P, P], dtype_mm)
        make_identity(nc, ident)

        # single PSUM pool, tags shared across stages (8 banks, no barriers):
        #   A bufs=3: ps_a | sc | po     B bufs=2: ps_b | av1
        #   C bufs=2: tp | av2           Dd bufs=1: bc
        ps = ctx.enter_context(tc.tile_pool(name="ps", bufs=1, space="PSUM"))
        s1 = ctx.enter_context(tc.tile_pool(name="s1", bufs=4))
        s1small = ctx.enter_context(tc.tile_pool(name="s1small", bufs=6))

        ro_tiles = {}  # (t, "q"/"k") -> rope-applied bf16 tile awaiting transpose

        def emit_transposes(t):
            for name, dst in (("q", qT_sb), ("k", kT_sb)):
                ro = ro_tiles.pop((t, name))
                for h in range(HPC):
                    tp = ps.tile([P, P], BF16, tag="C", bufs=2, name=f"tp{t}{name}{h}")
                    nc.tensor.transpose(tp[:], ro[:, ts(h, D)], ident[:])
                    if name == "q":
                        nc.scalar.activation(dst[h][:, ts(t, P)], tp[:],
                                             mybir.ActivationFunctionType.Copy)
                    else:
                        nc.vector.tensor_copy(dst[h][:, ts(t, P)], tp[:])

        for t in range(n_tiles):
            # two psum chunks: [q0,k0,q1,k1] (512) and [v0,v1] (256)
            ps_a = ps.tile([P, 4 * D], F32, tag="A", bufs=3, name=f"ps_a{t}")
            ps_b = ps.tile([P, 2 * D], F32, tag="B", bufs=2, name=f"ps_b{t}")
            for kk in range(K_IN):
                lhsT = xT_sb[:, kk, ts(t, P)]
                nc.tensor.matmul(ps_a[:], lhsT, wqkv_sb[:, kk, 0:4 * D],
                                 start=(kk == 0), stop=(kk == K_IN - 1))
                nc.tensor.matmul(ps_b[:], lhsT, wqkv_sb[:, kk, 4 * D:W_OUT],
                                 start=(kk == 0), stop=(kk == K_IN - 1))
            if emit_qk_bias:
                nc.tensor.matmul(ps_a[:], ones_row[:, :], bias_sb[:, 0:4 * D],
                                 start=False, stop=True)
                nc.tensor.matmul(ps_b[:], ones_row[:, :], bias_sb[:, 4 * D:W_OUT],
                                 start=False, stop=True)

            # free PSUM fast: one eviction (ACT), v-halves on DVE
            qk_raw = s1.tile([P, 4 * D], F32, tag="qk_raw")
            nc.scalar.activation(qk_raw[:], ps_a[:],
                                 mybir.ActivationFunctionType.Copy)
            pb4 = ps_b.rearrange("p (h x) -> p h x", x=D)
            nc.vector.tensor_copy(v1_all[:, t, :, 0:D // 2], pb4[:, :, 0:D // 2])
            nc.vector.tensor_copy(v2_all[:, t, :, :], pb4[:, :, D // 2:D])

            # LN stats for q0,k0,q1,k1
            stats = s1small.tile([P, 4, 6], F32, tag="stats")
            mv = s1small.tile([P, 4, 2], F32, tag="mv")
            rs = s1small.tile([P, 4], F32, tag="rs")
            for s in range(4):
                nc.vector.bn_stats(stats[:, s, :], qk_raw[:, ts(s, D)])
                nc.vector.bn_aggr(mv[:, s, :], stats[:, s, :])
            # rs = 1/sqrt(var+eps)
            sd = s1small.tile([P, 4], F32, tag="sd")
            nc.scalar.activation(sd[:, :], mv[:, :, 1],
                                 mybir.ActivationFunctionType.Sqrt,
                                 bias=eps_col[:])
            nc.vector.reciprocal(rs[:, :], sd[:, :])

            # bias = -(mu*rs) so ScalarE computes (x*rs + bias) = (x-mu)*rs
            negrs = s1small.tile([P, 4], F32, tag="negrs")
            nc.vector.tensor_scalar(negrs[:, :], rs[:, :], -1.0, None,
                                    mybir.AluOpType.mult)
            negmurs = s1small.tile([P, 4], F32, tag="negmurs")
            nc.vector.tensor_tensor(negmurs[:, :], mv[:, :, 0], negrs[:, :],
                                    mybir.AluOpType.mult)
            qn = s1.tile([P, HPC * D], BF16, tag="qn")
            kn = s1.tile([P, HPC * D], BF16, tag="kn")
            for h in range(HPC):
                nc.scalar.activation(qn[:, ts(h, D)], qk_raw[:, ts(2 * h, D)],
                                     mybir.ActivationFunctionType.Identity,
                                     bias=negmurs[:, 2 * h:2 * h + 1],
                                     scale=rs[:, 2 * h:2 * h + 1])
                nc.scalar.activation(kn[:, ts(h, D)], qk_raw[:, ts(2 * h + 1, D)],
                                     mybir.ActivationFunctionType.Identity,
                                     bias=negmurs[:, 2 * h + 1:2 * h + 2],
                                     scale=rs[:, 2 * h + 1:2 * h + 2])

            # rope: a = qn*cos (DVE); b = halves-swapped qn * sign-folded sin
            # (GPSIMD, sign baked into the table host-side); sum on DVE.
            for (xn, rt, name) in ((qn, ropeq_sb[:, t, :], "q"), (kn, ropek_sb[:, t, :], "k")):
                x4 = xn.rearrange("p (s x) -> p s x", x=D)
                a = s1.tile([P, HPC * D], BF16, tag=f"a{name}")
                bb = s1.tile([P, HPC * D], BF16, tag=f"b{name}")
                b4 = bb.rearrange("p (s x) -> p s x", x=D)
                sinS = rt[:, HPC * D:2 * HPC * D].rearrange("p (s x) -> p s x", x=D)
                nc.vector.tensor_tensor(a[:], xn[:], rt[:, 0:HPC * D],
                                        mybir.AluOpType.mult)
                nc.gpsimd.tensor_tensor(b4[:, :, 0:D // 2], x4[:, :, D // 2:D],
                                        sinS[:, :, 0:D // 2], mybir.AluOpType.mult)
                nc.gpsimd.tensor_tensor(b4[:, :, D // 2:D], x4[:, :, 0:D // 2],
                                        sinS[:, :, D // 2:D], mybir.AluOpType.mult)
                ro = s1.tile([P, HPC * D], dtype_mm, tag=f"ro{name}")
                eng = nc.vector if name == "q" else nc.gpsimd
                eng.tensor_tensor(ro[:], a[:], bb[:], mybir.AluOpType.add)
                ro_tiles[(t, name)] = ro

            # PE transposes, software-pipelined 2 tiles back
            if t >= 2:
                emit_transposes(t - 2)
        emit_transposes(n_tiles - 2)
        emit_transposes(n_tiles - 1)

        # ---------- stage 2+3: attention + out-projection, sw-pipelined ----------
        probs = ctx.enter_context(tc.tile_pool(name="probs", bufs=6))
        s2 = ctx.enter_context(tc.tile_pool(name="s2", bufs=3))
        s3 = ctx.enter_context(tc.tile_pool(name="s3", bufs=3))

        av_tiles = {}

        def emit_block(qi, h, mid=None):
            """scores + exp + fused AV/sums over all k tiles for (qi, h)."""
            av1 = ps.tile([D // 2 + 1, qc], F32, tag="B", bufs=2, name=f"av1_{qi}_{h}")
            av2 = ps.tile([D // 2, qc], F32, tag="C", bufs=2, name=f"av2_{qi}_{h}")
            for kt in range(n_tiles):
                if kt == 6 and mid is not None:
                    mid()
                sc = ps.tile([P, qc], F32, tag="A", bufs=3, name=f"sc{qi}{h}{kt}")
                nc.tensor.matmul(sc[:], kT_sb[h][:, ts(kt, P)],
                                 qT_sb[h][:, ds(qi * qc, qc)],
                                 start=True, stop=True)
                pr = probs.tile([P, qc], dtype_mm, tag="pr")
                nc.scalar.activation(pr[:], sc[:],
                                     mybir.ActivationFunctionType.Exp)
                nc.tensor.matmul(av1[:], v1_all[:, kt, h, :], pr[:],
                                 start=(kt == 0), stop=(kt == n_tiles - 1))
                nc.tensor.matmul(av2[:], v2_all[:, kt, h, :], pr[:],
                                 start=(kt == 0), stop=(kt == n_tiles - 1))
            av_tiles[(qi, h)] = (av1, av2)

        def emit_normalize(qi, h):
            """recip(sums row) -> broadcast -> scale both AV halves -> avn."""
            av1, av2 = av_tiles.pop((qi, h))
            rcp = s2.tile([1, qc], F32, tag="rcp")
            nc.vector.reciprocal(rcp[:], av1[D // 2:D // 2 + 1, :])
            bc = ps.tile([P, qc], F32, tag="Dd", bufs=1, name=f"bc{qi}{h}")
            nc.tensor.matmul(bc[:], ones_row[:], rcp[:], start=True, stop=True)
            bc_sb = s2.tile([P, qc], F32, tag="bc_sb")
            nc.vector.tensor_copy(bc_sb[:], bc[:])
            nc.vector.tensor_tensor(avn_sb[h][0:D // 2, :], av1[0:D // 2, :],
                                    bc_sb[0:D // 2, :], mybir.AluOpType.mult)
            nc.vector.tensor_tensor(avn_sb[h][D // 2:D, :], av2[:, :],
                                    bc_sb[D // 2:D, :], mybir.AluOpType.mult)

        def emit_proj(qi):
            """out-projection + store for this q chunk's tok tiles."""
            for ti in range(tpq):
                t = qi * tpq + ti
                for c in range(DIM // QC):
                    po = ps.tile([P, QC], F32, tag="A", bufs=3, name=f"po{t}{c}")
                    for h in range(HPC):
                        nc.tensor.matmul(po[:], avn_sb[h][:, ts(ti, P)],
                                         wout_sb[:, h, ts(c, QC)],
                                         start=(h == 0), stop=(h == HPC - 1))
                    ot = s3.tile([P, QC], F32, tag="ot")
                    nc.vector.tensor_copy(ot[:], po[:])
                    nc.scalar.dma_start(out[ts(t, P), ts(c, QC)], ot[:])

        blocks = [(qi, h) for qi in range(n_qc) for h in range(HPC)]
        for i, (qi, h) in enumerate(blocks):
            if i >= 1:
                pqi, ph = blocks[i - 1]
                emit_block(qi, h, mid=lambda p=(pqi, ph): emit_normalize(*p))
                if ph == HPC - 1:
                    emit_proj(pqi)
            else:
                emit_block(qi, h)
        emit_normalize(*blocks[-1])
        emit_proj(blocks[-1][0])

    return nc


# ---------------- host side ----------------

def _prep_core_inputs(x, Wqkv_w, Wqkv_b, q_gamma, k_gamma, out_w,
                      rope_cos, rope_sin, n_tok=N):
    """Build the 8 per-core input dicts (numpy, host-side sharding)."""
    bf = ml_dtypes.bfloat16
    scale = 1.0 / math.sqrt(D)
    # even-first permutation of head_dim and the rope partner map
    perm = np.concatenate([np.arange(0, D, 2), np.arange(1, D, 2)])
    partner = np.concatenate([np.arange(0, D, 2) + 1, np.arange(1, D, 2) - 1])
    # tables in permuted space; gamma folded in; q side also gets 1/sqrt(d)
    cosP = rope_cos[:, perm]
    sinP = rope_sin[:, perm]
    gq, gk = q_gamma, k_gamma
    cos_q = (cosP * gq[perm][None, :]) * scale
    sin_q = (sinP * gq[partner][None, :]) * scale
    cos_k = cosP * gk[perm][None, :]
    sin_k = sinP * gk[partner][None, :]
    # per-tile tables hold both heads side by side: [cos|cos|sin|sin]
    # fold rotate-half's sign into the sin tables: b[j<64] = q[j+64]*(-sin[j]),
    # b[j>=64] = q[j-64]*(+sin[j]) -- the device then only swaps halves via APs
    sgn = np.concatenate([-np.ones(D // 2, np.float32), np.ones(D // 2, np.float32)])
    sin_qS = sin_q * sgn[None, :]
    sin_kS = sin_k * sgn[None, :]
    ropeq = np.concatenate([cos_q, cos_q, sin_qS, sin_qS], axis=1).astype(bf)
    ropek = np.concatenate([cos_k, cos_k, sin_kS, sin_kS], axis=1).astype(bf)

    Wr = Wqkv_w.reshape(3, HEADS, D, DIM)
    in_maps = []
    for c in range(N_CORES):
        b = c // 4
        hs = [2 * (c % 4), 2 * (c % 4) + 1]
        xT = np.ascontiguousarray(x[b, :n_tok].T).astype(bf)
        blocks = []
        for h in hs:
            blocks.append(Wr[0, h][perm].T)  # q, dim-permuted  [DIM,128]
            blocks.append(Wr[1, h][perm].T)  # k, dim-permuted
        for h in hs:
            blocks.append(Wr[2, h].T)        # v, natural
        wqkv = np.concatenate(blocks, axis=1).astype(bf)  # [DIM, 768]
        wout = np.concatenate(
            [out_w[:, h * D:(h + 1) * D].T for h in hs], axis=0).astype(bf)  # [256,DIM]
        in_maps.append({
            "xT": xT,
            "wqkv": np.ascontiguousarray(wqkv),
            "wout": np.ascontiguousarray(wout),
            "ropeq": ropeq[:n_tok],
            "ropek": ropek[:n_tok],
        })
    return in_maps


def kernel(x, Wqkv_w, Wqkv_b, q_gamma, q_beta, k_gamma, k_beta,
           out_w, out_b, rope_cos, rope_sin, trace=False, tmpdir=None):
    x = np.asarray(x, np.float32)
    Wqkv_w = np.asarray(Wqkv_w, np.float32)
    Wqkv_b = np.asarray(Wqkv_b, np.float32)
    q_gamma = np.asarray(q_gamma, np.float32)
    q_beta = np.asarray(q_beta, np.float32)
    k_gamma = np.asarray(k_gamma, np.float32)
    k_beta = np.asarray(k_beta, np.float32)
    out_w = np.asarray(out_w, np.float32)
    out_b = np.asarray(out_b, np.float32)
    rope_cos = np.asarray(rope_cos, np.float32)
    rope_sin = np.asarray(rope_sin, np.float32)

    assert np.allclose(q_beta, 0) and np.allclose(k_beta, 0), \
        "nonzero q/k layernorm beta not supported by this kernel build"
    emit_qk_bias = not (np.allclose(Wqkv_b[:DIM], 0) and np.allclose(Wqkv_b[DIM:2 * DIM], 0))

    nc = bacc.Bacc("TRN2", target_bir_lowering=False, debug=False,
                   num_devices=N_CORES)
    build_core_graph(nc, n_tok=N, emit_qk_bias=emit_qk_bias)
    nc.compile()

    in_maps = _prep_core_inputs(x, Wqkv_w, Wqkv_b, q_gamma, k_gamma,
                                out_w, rope_cos, rope_sin)
    if emit_qk_bias:
        for c in range(N_CORES):
            hs = [2 * (c % 4), 2 * (c % 4) + 1]
            bq = Wqkv_b[:DIM].reshape(HEADS, D)
            bk = Wqkv_b[DIM:2 * DIM].reshape(HEADS, D)
            perm = np.concatenate([np.arange(0, D, 2), np.arange(1, D, 2)])
            blocks = [np.zeros(0, np.float32)]
            for h in hs:
                blocks += [bq[h][perm], bk[h][perm]]
            blocks += [np.zeros(2 * D, np.float32)]
            in_maps[c]["bqkv"] = np.concatenate(blocks)[None, :].astype(np.float32)

    res = run_bass_kernel_spmd(nc, in_maps, core_ids=list(range(N_CORES)),
                               trace=trace, tmpdir=tmpdir)
    partials = [np.asarray(r["out"], np.float32) for r in res.results]

    # host gather: sum the 4 head-group partials per batch; fold v-bias + out_b
    bv = Wqkv_b[2 * DIM:]
    bias_row = out_b + bv @ out_w.T  # [DIM]
    outp = np.empty((B, N, DIM), np.float32)
    for b in range(B):
        outp[b] = sum(partials[4 * b:4 * b + 4]) + bias_row[None, :]
    kernel.last_exec_time_ns = res.exec_time_ns
    return outp
